# revision 19
# baseline (speedup 1.0000x reference)
"""BlockAttention TRN2 Bass kernel.

Problem (hardcoded): x [4, 4096, 1024] fp32; wq/wk/wv/wo [1024, 1024];
bq/bk/bv/bo [1024]; block_size 256. Output [4, 8192, 1024]:
per 256-token block g: rows [512g, 512g+256) = softmax(Q_g K_g^T / 32) V_g @ wo,
rows [512g+256, 512g+512) = softmax(Q_g K_{g-1}^T / 32) V_{g-1} @ wo (block 0
attends to itself), all + bo.

Sharding: 8 cores = 4 batches x 2 sequence halves (8 q-blocks each). Each core
gets x^T for its 9 kv blocks (prev + 8 own; block 0's "prev" is itself), all
weights, and writes out^T [1024, 4096] for its 4096 output rows.

Per-core algorithm (all matmuls in fp32r = full-rate ~tf32 precision):
  - Q^T/K^T/V^T = W^T x^T per block (contraction over d_in on partitions).
  - VW = V @ wo per kv block, computed once, reused by the local attention of
    block g and the cross attention of block g+1 (halves the out-proj flops:
    out = P @ (V @ wo)).
  - S^T [keys, queries] = K Q^T directly (no transposes anywhere); softmax over
    the partition (key) dim: exp on ScalarE, key-sums via ones-vector matmul,
    reciprocal on VectorE, broadcast back via rank-1 matmul, normalize in-place.
  - out^T = VW^T P^T accumulated in PSUM, DMA'd straight to DRAM.

bo is added on the host (exact, zero-cost on device).
"""

import numpy as np
from contextlib import ExitStack

import concourse.bass as bass
import concourse.mybir as mybir
import concourse.tile as tile
from concourse import bacc, bass_utils

D = 1024
BS = 256
NBQ = 8  # q-blocks per core
NKV = NBQ + 1  # kv blocks in xt (prev + own 8)
TKV = NKV * BS  # 2304
DS = D // 128  # 8 subtiles of the feature dim
F32 = mybir.dt.float32
F32R = mybir.dt.float32r
SCALE = 1.0 / 32.0  # 1/sqrt(D)

_CACHED_NC = None


def _build():
    nc = bacc.Bacc("TRN2", target_bir_lowering=False, debug=False, num_devices=8)
    xt = nc.dram_tensor("xt", [D, TKV], F32, kind="ExternalInput").ap()
    w_ap = {
        n: nc.dram_tensor(n, [D, D], F32, kind="ExternalInput").ap()
        for n in ("wq", "wk", "wv", "wo")
    }
    b_ap = {
        n: nc.dram_tensor(n, [128, DS], F32, kind="ExternalInput").ap()
        for n in ("bq", "bk", "bv")
    }
    ones2d = nc.dram_tensor("ones2d", [128, 128], F32, kind="ExternalInput").ap()
    outt = nc.dram_tensor("outt", [D, NBQ * 2 * BS], F32, kind="ExternalOutput").ap()

    with (
        tile.TileContext(nc) as tc,
        ExitStack() as ctx,
        nc.allow_low_precision(reason="fp32r (tf32-like) matmul inputs by design"),
    ):
        wp = ctx.enter_context(tc.tile_pool(name="wp", bufs=1))
        cp = ctx.enter_context(tc.tile_pool(name="cp", bufs=1))
        xp = ctx.enter_context(tc.tile_pool(name="xp", bufs=2))
        qp = ctx.enter_context(tc.tile_pool(name="qp", bufs=1))
        kp = ctx.enter_context(tc.tile_pool(name="kp", bufs=2))
        vp = ctx.enter_context(tc.tile_pool(name="vp", bufs=1))
        wvp = ctx.enter_context(tc.tile_pool(name="wvp", bufs=2))
        pp = ctx.enter_context(tc.tile_pool(name="pp", bufs=2))
        rp = ctx.enter_context(tc.tile_pool(name="rp", bufs=2))
        op_sb = ctx.enter_context(tc.tile_pool(name="op_sb", bufs=4))
        PSUM = bass.MemorySpace.PSUM
        ps_mm = ctx.enter_context(tc.tile_pool(name="ps_mm", bufs=3, space=PSUM))
        ps_st = ctx.enter_context(tc.tile_pool(name="ps_st", bufs=2, space=PSUM))
        ps_op = ctx.enter_context(tc.tile_pool(name="ps_op", bufs=3, space=PSUM))

        # Weights as matmul lhsT: [d_in, d_out], d_in-subtile k at cols [D*k, D*(k+1))
        # Loaded lazily (DMA packets drain in emission order — a weight
        # emitted before the compute that needs it, and no earlier, keeps
        # the startup transient minimal).
        w_sb = {}

        def load_w(n):
            t = wp.tile([128, DS * D], F32R, tag=n)
            for s in range(DS):
                nc.sync.dma_start(
                    t[:, D * s : D * (s + 1)],
                    w_ap[n][128 * s : 128 * (s + 1), :].bitcast(F32R),
                )
            w_sb[n] = t

        b_sb = {}
        for n in ("bq", "bk", "bv"):
            t = cp.tile([128, DS], F32, tag=n)
            nc.sync.dma_start(t[:], b_ap[n])
            b_sb[n] = t
        ones_sb = cp.tile([128, 128], F32R, tag="ones")
        nc.sync.dma_start(ones_sb[:], ones2d.bitcast(F32R))

        def load_x(blk):
            # x^T block: [128, DS*BS], d-subtile s at cols [BS*s, BS*(s+1)).
            # gpsimd queue: keeps the sync queue free for weights + outputs.
            t = xp.tile([128, DS * BS], F32R, tag="x")
            for s in range(DS):
                nc.gpsimd.dma_start(
                    t[:, BS * s : BS * (s + 1)],
                    xt[128 * s : 128 * (s + 1), BS * blk : BS * (blk + 1)].bitcast(
                        F32R
                    ),
                )
            return t

        def proj_T(xtile, wname, bname, tag, pool):
            # (W^T x^T)[d_out, tok]: [128, DS*BS], d_out-subtile m at cols [BS*m, ..)
            dst = pool.tile([128, DS * BS], F32R, tag=tag)
            for m in range(DS):
                pst = ps_mm.tile([128, BS], F32, tag="mm")
                for k in range(DS):
                    nc.tensor.matmul(
                        pst[:],
                        w_sb[wname][:, D * k + 128 * m : D * k + 128 * (m + 1)],
                        xtile[:, BS * k : BS * (k + 1)],
                        start=(k == 0),
                        stop=(k == DS - 1),
                    )
                nc.scalar.activation(
                    dst[:, BS * m : BS * (m + 1)],
                    pst[:],
                    mybir.ActivationFunctionType.Identity,
                    bias=b_sb[bname][:, m : m + 1],
                )
            return dst

        def vw_proj(vt):
            # (V @ wo)[tok, d_out]: [128, 2*D], token-subtile ts at cols [D*ts, ..)
            dst = wvp.tile([128, 2 * D], F32R, tag="vw")
            for ts in range(2):
                for half in range(2):
                    pst = ps_mm.tile([128, 512], F32, tag="mm")
                    for k in range(DS):
                        nc.tensor.matmul(
                            pst[:],
                            vt[:, BS * k + 128 * ts : BS * k + 128 * (ts + 1)],
                            w_sb["wo"][:, D * k + 512 * half : D * k + 512 * (half + 1)],
                            start=(k == 0),
                            stop=(k == DS - 1),
                        )
                    nc.vector.tensor_copy(
                        dst[:, D * ts + 512 * half : D * ts + 512 * (half + 1)], pst[:]
                    )
            return dst

        # Attention in three emission phases so local/cross interleave on the
        # in-order PE stream: S^T matmuls for both halves first (ACT exp of the
        # first hides under the second's matmuls), then both normalizations,
        # then both output projections.
        def attend_scores(qt, kt):
            # expS^T = exp(K Q^T / 32), unnormalized
            ptile = pp.tile([128, 2 * BS], F32R, tag="pt")
            for ks in range(2):
                pst = ps_st.tile([128, BS], F32, tag="st")
                for k in range(DS):
                    nc.tensor.matmul(
                        pst[:],
                        kt[:, BS * k + 128 * ks : BS * k + 128 * (ks + 1)],
                        qt[:, BS * k : BS * (k + 1)],
                        start=(k == 0),
                        stop=(k == DS - 1),
                    )
                nc.scalar.activation(
                    ptile[:, BS * ks : BS * (ks + 1)],
                    pst[:],
                    mybir.ActivationFunctionType.Exp,
                    scale=SCALE,
                )
            return ptile

        def attend_norm(ptile):
            # Broadcasted column-sums in one matmul: ones[k,128].T @ expS^T
            # gives the key-sum in every output row; 128-lane reciprocal.
            # The normalization itself is deferred to attend_out's PSUM->SBUF
            # copy (diag scaling commutes with the V@wo projection), keeping
            # the 1.7us reciprocal entirely off the PE critical path.
            bc = ps_st.tile([128, BS], F32, tag="st")
            for ks in range(2):
                nc.tensor.matmul(
                    bc[:],
                    ones_sb[:],
                    ptile[:, BS * ks : BS * (ks + 1)],
                    start=(ks == 0),
                    stop=(ks == 1),
                )
            rc = rp.tile([128, BS], F32R, tag="rc")
            nc.vector.reciprocal(rc[:], bc[:])
            return rc

        def attend_out(ptile, rc, vw, t, h):
            col0 = 2 * BS * t + BS * h
            for m in range(DS):
                pso = ps_op.tile([128, BS], F32, tag="op")
                for ks in range(2):
                    nc.tensor.matmul(
                        pso[:],
                        vw[:, D * ks + 128 * m : D * ks + 128 * (m + 1)],
                        ptile[:, BS * ks : BS * (ks + 1)],
                        start=(ks == 0),
                        stop=(ks == 1),
                    )
                ostage = op_sb.tile([128, BS], F32, tag="os")
                nc.vector.tensor_mul(ostage[:], pso[:], rc[:])
                nc.sync.dma_start(
                    outt[128 * m : 128 * (m + 1), col0 : col0 + BS], ostage[:]
                )

        x_prev = load_x(0)
        load_w("wk")
        kt_prev = proj_T(x_prev, "wk", "bk", "kt", kp)
        load_w("wv")
        vt = proj_T(x_prev, "wv", "bv", "vt", vp)
        load_w("wo")
        vw_prev = vw_proj(vt)
        load_w("wq")
        for t in range(NBQ):
            # kv-projections first: at startup this matches the weight DMA
            # arrival order (wk, wv, wo, then wq) with zero stalls.
            x_cur = load_x(t + 1)
            kt_cur = proj_T(x_cur, "wk", "bk", "kt", kp)
            vt = proj_T(x_cur, "wv", "bv", "vt", vp)
            vw_cur = vw_proj(vt)
            qt = proj_T(x_cur, "wq", "bq", "qt", qp)
            p_loc = attend_scores(qt, kt_cur)
            p_cross = attend_scores(qt, kt_prev)
            rc_loc = attend_norm(p_loc)
            attend_out(p_loc, rc_loc, vw_cur, t, 0)  # local (own block)
            rc_cross = attend_norm(p_cross)
            attend_out(p_cross, rc_cross, vw_prev, t, 1)  # cross (prev block)
            kt_prev, vw_prev = kt_cur, vw_cur

    nc.compile()
    return nc


def _get_nc():
    global _CACHED_NC
    if _CACHED_NC is None:
        _CACHED_NC = _build()
    return _CACHED_NC


def _make_in_maps(x, wq, bq, wk, bk, wv, bv, wo):
    base = {
        "wq": np.ascontiguousarray(wq, np.float32),
        "wk": np.ascontiguousarray(wk, np.float32),
        "wv": np.ascontiguousarray(wv, np.float32),
        "wo": np.ascontiguousarray(wo, np.float32),
        "bq": np.ascontiguousarray(bq.reshape(DS, 128).T, np.float32),
        "bk": np.ascontiguousarray(bk.reshape(DS, 128).T, np.float32),
        "bv": np.ascontiguousarray(bv.reshape(DS, 128).T, np.float32),
        "ones2d": np.ones((128, 128), np.float32),
    }
    in_maps = []
    for c in range(8):
        b, t = c // 2, c % 2
        if t == 0:
            xkv = np.concatenate([x[b, 0:BS], x[b, 0 : NBQ * BS]], axis=0)
        else:
            xkv = x[b, NBQ * BS - BS : 2 * NBQ * BS]
        in_maps.append(
            {**base, "xt": np.ascontiguousarray(xkv.T, dtype=np.float32)}
        )
    return in_maps


def _assemble(results, bo):
    out = np.empty((4, 2 * NBQ * 2 * BS, D), np.float32)
    for c in range(8):
        b, t = c // 2, c % 2
        seg = NBQ * 2 * BS  # 4096 output rows per core
        out[b, seg * t : seg * (t + 1), :] = results[c]["outt"].T
    out += np.asarray(bo, np.float32).reshape(1, 1, D)
    return out


def run(x, wq, bq, wk, bk, wv, bv, wo, bo, trace=False):
    nc = _get_nc()
    in_maps = _make_in_maps(x, wq, bq, wk, bk, wv, bv, wo)
    res = bass_utils.run_bass_kernel_spmd(
        nc, in_maps, core_ids=list(range(8)), trace=trace
    )
    return _assemble(res.results, bo), res


def kernel(x, wq, bq, wk, bk, wv, bv, wo, bo, block_size):
    assert int(block_size) == BS
    x = np.asarray(x, np.float32)
    assert x.shape == (4, 2 * NBQ * BS, D), x.shape
    args = [np.asarray(a, np.float32) for a in (wq, bq, wk, bk, wv, bv, wo, bo)]
    wq, bq, wk, bk, wv, bv, wo, bo = args
    out, _ = run(x, wq, bq, wk, bk, wv, bv, wo, bo, trace=False)
    return out


# revision 22
# speedup vs baseline: 1.0299x; 1.0299x over previous
"""BlockAttention TRN2 Bass kernel.

Problem (hardcoded): x [4, 4096, 1024] fp32; wq/wk/wv/wo [1024, 1024];
bq/bk/bv/bo [1024]; block_size 256. Output [4, 8192, 1024]:
per 256-token block g: rows [512g, 512g+256) = softmax(Q_g K_g^T / 32) V_g @ wo,
rows [512g+256, 512g+512) = softmax(Q_g K_{g-1}^T / 32) V_{g-1} @ wo (block 0
attends to itself), all + bo.

Sharding: 8 cores = 4 batches x 2 sequence halves (8 q-blocks each). Each core
gets x^T for its 9 kv blocks (prev + 8 own; block 0's "prev" is itself), all
weights, and writes out^T [1024, 4096] for its 4096 output rows.

Per-core algorithm (all matmuls in fp32r = full-rate ~tf32 precision):
  - Q^T/K^T/V^T = W^T x^T per block (contraction over d_in on partitions).
  - VW = V @ wo per kv block, computed once, reused by the local attention of
    block g and the cross attention of block g+1 (halves the out-proj flops:
    out = P @ (V @ wo)).
  - S^T [keys, queries] = K Q^T directly (no transposes anywhere); softmax over
    the partition (key) dim: exp on ScalarE, key-sums via ones-vector matmul,
    reciprocal on VectorE, broadcast back via rank-1 matmul, normalize in-place.
  - out^T = VW^T P^T accumulated in PSUM, DMA'd straight to DRAM.

bo is added on the host (exact, zero-cost on device).
"""

import numpy as np
from contextlib import ExitStack

import concourse.bass as bass
import concourse.mybir as mybir
import concourse.tile as tile
from concourse import bacc, bass_utils

D = 1024
BS = 256
NBQ = 8  # q-blocks per core
NKV = NBQ + 1  # kv blocks in xt (prev + own 8)
TKV = NKV * BS  # 2304
DS = D // 128  # 8 subtiles of the feature dim
F32 = mybir.dt.float32
F32R = mybir.dt.float32r
SCALE = 1.0 / 32.0  # 1/sqrt(D)

_CACHED_NC = None


def _build():
    nc = bacc.Bacc("TRN2", target_bir_lowering=False, debug=False, num_devices=8)
    xt = nc.dram_tensor("xt", [D, TKV], F32, kind="ExternalInput").ap()
    w_ap = {
        n: nc.dram_tensor(n, [D, D], F32, kind="ExternalInput").ap()
        for n in ("wq", "wk", "wv", "wo")
    }
    b_ap = {
        n: nc.dram_tensor(n, [128, DS], F32, kind="ExternalInput").ap()
        for n in ("bq", "bk", "bv")
    }
    ones2d = nc.dram_tensor("ones2d", [128, 128], F32, kind="ExternalInput").ap()
    outt = nc.dram_tensor("outt", [D, NBQ * 2 * BS], F32, kind="ExternalOutput").ap()

    with (
        tile.TileContext(nc) as tc,
        ExitStack() as ctx,
        nc.allow_low_precision(reason="fp32r (tf32-like) matmul inputs by design"),
    ):
        wp = ctx.enter_context(tc.tile_pool(name="wp", bufs=1))
        cp = ctx.enter_context(tc.tile_pool(name="cp", bufs=1))
        xp = ctx.enter_context(tc.tile_pool(name="xp", bufs=2))
        qp = ctx.enter_context(tc.tile_pool(name="qp", bufs=1))
        kp = ctx.enter_context(tc.tile_pool(name="kp", bufs=2))
        vp = ctx.enter_context(tc.tile_pool(name="vp", bufs=1))
        wvp = ctx.enter_context(tc.tile_pool(name="wvp", bufs=2))
        pp = ctx.enter_context(tc.tile_pool(name="pp", bufs=2))
        rp = ctx.enter_context(tc.tile_pool(name="rp", bufs=2))
        op_sb = ctx.enter_context(tc.tile_pool(name="op_sb", bufs=4))
        PSUM = bass.MemorySpace.PSUM
        ps_mm = ctx.enter_context(tc.tile_pool(name="ps_mm", bufs=2, space=PSUM))
        ps_st = ctx.enter_context(tc.tile_pool(name="ps_st", bufs=2, space=PSUM))
        ps_op = ctx.enter_context(tc.tile_pool(name="ps_op", bufs=4, space=PSUM))

        # Weights as matmul lhsT: [d_in, d_out], d_in-subtile k at cols [D*k, D*(k+1))
        # Loaded lazily (DMA packets drain in emission order — a weight
        # emitted before the compute that needs it, and no earlier, keeps
        # the startup transient minimal).
        w_sb = {}

        def load_w(n):
            t = wp.tile([128, DS * D], F32R, tag=n)
            for s in range(DS):
                nc.sync.dma_start(
                    t[:, D * s : D * (s + 1)],
                    w_ap[n][128 * s : 128 * (s + 1), :].bitcast(F32R),
                )
            w_sb[n] = t

        b_sb = {}
        for n in ("bq", "bk", "bv"):
            t = cp.tile([128, DS], F32, tag=n)
            nc.sync.dma_start(t[:], b_ap[n])
            b_sb[n] = t
        ones_sb = cp.tile([128, 128], F32R, tag="ones")
        nc.sync.dma_start(ones_sb[:], ones2d.bitcast(F32R))

        def load_x(blk):
            # x^T block: [128, DS*BS], d-subtile s at cols [BS*s, BS*(s+1)).
            # gpsimd queue: keeps the sync queue free for weights + outputs.
            t = xp.tile([128, DS * BS], F32R, tag="x")
            for s in range(DS):
                nc.gpsimd.dma_start(
                    t[:, BS * s : BS * (s + 1)],
                    xt[128 * s : 128 * (s + 1), BS * blk : BS * (blk + 1)].bitcast(
                        F32R
                    ),
                )
            return t

        def proj_T(xtile, wname, bname, tag, pool):
            # (W^T x^T)[d_out, tok]: [128, DS*BS], d_out-subtile m at cols [BS*m, ..)
            dst = pool.tile([128, DS * BS], F32R, tag=tag)
            for m in range(DS):
                pst = ps_mm.tile([128, BS], F32, tag="mm")
                for k in range(DS):
                    nc.tensor.matmul(
                        pst[:],
                        w_sb[wname][:, D * k + 128 * m : D * k + 128 * (m + 1)],
                        xtile[:, BS * k : BS * (k + 1)],
                        start=(k == 0),
                        stop=(k == DS - 1),
                    )
                nc.scalar.activation(
                    dst[:, BS * m : BS * (m + 1)],
                    pst[:],
                    mybir.ActivationFunctionType.Identity,
                    bias=b_sb[bname][:, m : m + 1],
                )
            return dst

        def vw_proj(vt):
            # (V @ wo)[tok, d_out]: [128, 2*D], token-subtile ts at cols [D*ts, ..)
            dst = wvp.tile([128, 2 * D], F32R, tag="vw")
            for ts in range(2):
                for half in range(2):
                    pst = ps_mm.tile([128, 512], F32, tag="mm")
                    for k in range(DS):
                        nc.tensor.matmul(
                            pst[:],
                            vt[:, BS * k + 128 * ts : BS * k + 128 * (ts + 1)],
                            w_sb["wo"][:, D * k + 512 * half : D * k + 512 * (half + 1)],
                            start=(k == 0),
                            stop=(k == DS - 1),
                        )
                    nc.vector.tensor_copy(
                        dst[:, D * ts + 512 * half : D * ts + 512 * (half + 1)], pst[:]
                    )
            return dst

        # Attention in three emission phases so local/cross interleave on the
        # in-order PE stream: S^T matmuls for both halves first (ACT exp of the
        # first hides under the second's matmuls), then both normalizations,
        # then both output projections.
        def attend_scores(qt, kt):
            # expS^T = exp(K Q^T / 32), unnormalized
            ptile = pp.tile([128, 2 * BS], F32R, tag="pt")
            for ks in range(2):
                pst = ps_st.tile([128, BS], F32, tag="st")
                for k in range(DS):
                    nc.tensor.matmul(
                        pst[:],
                        kt[:, BS * k + 128 * ks : BS * k + 128 * (ks + 1)],
                        qt[:, BS * k : BS * (k + 1)],
                        start=(k == 0),
                        stop=(k == DS - 1),
                    )
                nc.scalar.activation(
                    ptile[:, BS * ks : BS * (ks + 1)],
                    pst[:],
                    mybir.ActivationFunctionType.Exp,
                    scale=SCALE,
                )
            return ptile

        def attend_norm(ptile):
            # Broadcasted column-sums in one matmul: ones[k,128].T @ expS^T
            # gives the key-sum in every output row; 128-lane reciprocal.
            # The normalization itself is deferred to attend_out's PSUM->SBUF
            # copy (diag scaling commutes with the V@wo projection), keeping
            # the 1.7us reciprocal entirely off the PE critical path.
            bc = ps_st.tile([128, BS], F32, tag="st")
            for ks in range(2):
                nc.tensor.matmul(
                    bc[:],
                    ones_sb[:],
                    ptile[:, BS * ks : BS * (ks + 1)],
                    start=(ks == 0),
                    stop=(ks == 1),
                )
            rc = rp.tile([128, BS], F32R, tag="rc")
            nc.vector.reciprocal(rc[:], bc[:])
            return rc

        def attend_out(ptile, rc, vw, t, h):
            # Two d_out m-tiles share one [128,512] PSUM bank so only 4 slots
            # cycle per attend (= ps_op bufs): the PE never waits on the DVE
            # normalize-muls inside an attend; they drain under later phases.
            col0 = 2 * BS * t + BS * h
            for mp in range(DS // 2):
                pso = ps_op.tile([128, 2 * BS], F32, tag="op")
                for sub in range(2):
                    m = 2 * mp + sub
                    for ks in range(2):
                        nc.tensor.matmul(
                            pso[:, BS * sub : BS * (sub + 1)],
                            vw[:, D * ks + 128 * m : D * ks + 128 * (m + 1)],
                            ptile[:, BS * ks : BS * (ks + 1)],
                            start=(ks == 0),
                            stop=(ks == 1),
                        )
                for sub in range(2):
                    m = 2 * mp + sub
                    ostage = op_sb.tile([128, BS], F32, tag="os")
                    nc.vector.tensor_mul(
                        ostage[:], pso[:, BS * sub : BS * (sub + 1)], rc[:]
                    )
                    nc.sync.dma_start(
                        outt[128 * m : 128 * (m + 1), col0 : col0 + BS], ostage[:]
                    )

        # Prologue covers kv-blocks 0 and 1 plus q-block 0, ordered so each
        # phase's weight has arrived by the time the PE reaches it
        # (DMA queue order: wk, wv, wo, wq at ~300 GB/s).
        x0 = load_x(0)
        load_w("wk")
        x_cur = load_x(1)
        kt_prev = proj_T(x0, "wk", "bk", "kt", kp)
        kt_cur = proj_T(x_cur, "wk", "bk", "kt", kp)
        load_w("wv")
        vt = proj_T(x0, "wv", "bv", "vt", vp)
        load_w("wo")
        vw_prev = vw_proj(vt)
        vt = proj_T(x_cur, "wv", "bv", "vt", vp)
        vw_cur = vw_proj(vt)
        load_w("wq")
        qt = proj_T(x_cur, "wq", "bq", "qt", qp)
        p_loc = attend_scores(qt, kt_cur)
        p_cross = attend_scores(qt, kt_prev)
        rc_loc = attend_norm(p_loc)
        attend_out(p_loc, rc_loc, vw_cur, 0, 0)
        rc_cross = attend_norm(p_cross)
        attend_out(p_cross, rc_cross, vw_prev, 0, 1)
        kt_prev, vw_prev = kt_cur, vw_cur
        for t in range(1, NBQ):
            # kv-projections first: at startup this matches the weight DMA
            # arrival order (wk, wv, wo, then wq) with zero stalls.
            x_cur = load_x(t + 1)
            kt_cur = proj_T(x_cur, "wk", "bk", "kt", kp)
            vt = proj_T(x_cur, "wv", "bv", "vt", vp)
            vw_cur = vw_proj(vt)
            qt = proj_T(x_cur, "wq", "bq", "qt", qp)
            p_loc = attend_scores(qt, kt_cur)
            p_cross = attend_scores(qt, kt_prev)
            rc_loc = attend_norm(p_loc)
            attend_out(p_loc, rc_loc, vw_cur, t, 0)  # local (own block)
            rc_cross = attend_norm(p_cross)
            attend_out(p_cross, rc_cross, vw_prev, t, 1)  # cross (prev block)
            kt_prev, vw_prev = kt_cur, vw_cur

    nc.compile()
    return nc


def _get_nc():
    global _CACHED_NC
    if _CACHED_NC is None:
        _CACHED_NC = _build()
    return _CACHED_NC


def _make_in_maps(x, wq, bq, wk, bk, wv, bv, wo):
    base = {
        "wq": np.ascontiguousarray(wq, np.float32),
        "wk": np.ascontiguousarray(wk, np.float32),
        "wv": np.ascontiguousarray(wv, np.float32),
        "wo": np.ascontiguousarray(wo, np.float32),
        "bq": np.ascontiguousarray(bq.reshape(DS, 128).T, np.float32),
        "bk": np.ascontiguousarray(bk.reshape(DS, 128).T, np.float32),
        "bv": np.ascontiguousarray(bv.reshape(DS, 128).T, np.float32),
        "ones2d": np.ones((128, 128), np.float32),
    }
    in_maps = []
    for c in range(8):
        b, t = c // 2, c % 2
        if t == 0:
            xkv = np.concatenate([x[b, 0:BS], x[b, 0 : NBQ * BS]], axis=0)
        else:
            xkv = x[b, NBQ * BS - BS : 2 * NBQ * BS]
        in_maps.append(
            {**base, "xt": np.ascontiguousarray(xkv.T, dtype=np.float32)}
        )
    return in_maps


def _assemble(results, bo):
    out = np.empty((4, 2 * NBQ * 2 * BS, D), np.float32)
    for c in range(8):
        b, t = c // 2, c % 2
        seg = NBQ * 2 * BS  # 4096 output rows per core
        out[b, seg * t : seg * (t + 1), :] = results[c]["outt"].T
    out += np.asarray(bo, np.float32).reshape(1, 1, D)
    return out


def run(x, wq, bq, wk, bk, wv, bv, wo, bo, trace=False):
    nc = _get_nc()
    in_maps = _make_in_maps(x, wq, bq, wk, bk, wv, bv, wo)
    res = bass_utils.run_bass_kernel_spmd(
        nc, in_maps, core_ids=list(range(8)), trace=trace
    )
    return _assemble(res.results, bo), res


def kernel(x, wq, bq, wk, bk, wv, bv, wo, bo, block_size):
    assert int(block_size) == BS
    x = np.asarray(x, np.float32)
    assert x.shape == (4, 2 * NBQ * BS, D), x.shape
    args = [np.asarray(a, np.float32) for a in (wq, bq, wk, bk, wv, bv, wo, bo)]
    wq, bq, wk, bk, wv, bv, wo, bo = args
    out, _ = run(x, wq, bq, wk, bk, wv, bv, wo, bo, trace=False)
    return out
